# revision 1
# baseline (speedup 1.0000x reference)
"""Trainium2 Bass kernel for DGCRNNCell (nn_DGCRNNCell_21792664060192).

Computes, for each batch item b and head h over graph with N=199 nodes:
  feat   = einsum('nf,nm->mf', X[b], A*W[h])          (via featT = X^T-style chain)
  dense  = feat @ kernel[h] + bias1[h]
  mask   = softmax(dense - NEG*(1-A), axis=-1)        (adjacency-masked softmax)
  node   = mask @ X[b]
  out_h  = node @ T[h] + bias2[h]
  output[b] = concat([out_0..out_3 (r, 256)], mask_3 (r, 199))   -> (199, 455)

Sharding: pure data-parallel over batch (512 -> 64 per core x 8 cores).

Dataflow on device (per b), all matmul chains arranged so that no transpose
is ever needed (the contraction axis always lands on partitions):
  featT  (f=64, m)    = sum_n  Xb(n,f)^T ... lhsT=Xb chunk, rhs=AW[h] chunk
  denseT (c, r)       : lhsT=kernel[h](f,c-chunk), rhs=featT(f,r)
  expT   (c, r)       = exp(denseT) * EA[h]   where EA[h][c,r] = A[r,c]*e^{bias1[h,c]}
  nodeUT (65, r)      : lhsT=Xb_aug(c,65) (ones col -> row 64 = softmax denom s[r])
  outU   (r, 65h+j)   : lhsT=nodeUT(65, r-chunk), rhs=T_aug[h](65,65)
                        col 64 of each head block = s[r]; row 64 of T_aug = bias2
  out    (r, h*64+u)  = outU * (1/s[r])   (per-partition broadcast multiply)
Head-3 mask output is computed in (r, c) orientation directly:
  dense_rc: lhsT=featT_3(f, r-chunk), rhs=kernel[3](f, c); exp; * EAT3; * 1/s3.
"""

import numpy as np

import concourse.bass as bass
import concourse.mybir as mybir
import concourse.tile as tile
from concourse import bacc
from concourse.bass_utils import run_bass_kernel_spmd

B, N, F, U, H = 512, 199, 64, 64, 4
NCORES = 8
BPC = B // NCORES  # 64 batch items per core
P0 = 128
P1 = N - P0  # 71
FA = F + 1  # X augmented with ones column
OUTC = H * U + N  # 455
DT = mybir.dt.float32
AF = mybir.ActivationFunctionType
ALU = mybir.AluOpType

_CHUNKS = ((0, P0), (P0, P1))  # (offset, size) along the N(=c or r) axis


def _build_kernel(nc: bass.Bass, tc: "tile.TileContext", io: dict, bpc: int = BPC):
    import os
    from contextlib import ExitStack

    stage = int(os.environ.get("KSTAGE", "9"))

    Xa, AWc, K2, K3HI, EAc, EAT3, TA, O = (
        io["Xa"], io["AWc"], io["K2"], io["K3HI"], io["EAc"], io["EAT3"], io["TA"], io["O"],
    )

    def _b(name, default):
        return int(os.environ.get(name, str(default)))

    with ExitStack() as ctx:
        cpool = ctx.enter_context(tc.tile_pool(name="consts", bufs=1))
        xpool = ctx.enter_context(tc.tile_pool(name="xa", bufs=_b("XB", 3)))
        fspool = ctx.enter_context(tc.tile_pool(name="fs", bufs=_b("FSB", 2)))
        epool = ctx.enter_context(tc.tile_pool(name="expT", bufs=_b("EB", 2)))
        nspool = ctx.enter_context(tc.tile_pool(name="nS", bufs=_b("NSB", 2)))
        erpool = ctx.enter_context(tc.tile_pool(name="eR", bufs=_b("ERB", 2)))
        rpool = ctx.enter_context(tc.tile_pool(name="rec", bufs=_b("RB", 2)))
        opool = ctx.enter_context(tc.tile_pool(name="sO", bufs=_b("OB", 3)))
        pf = ctx.enter_context(
            tc.tile_pool(name="pfnu", bufs=_b("FNB", 2), space="PSUM")
        )
        pd = ctx.enter_context(
            tc.tile_pool(name="pdnu", bufs=_b("DTB", 2), space="PSUM")
        )
        po = ctx.enter_context(
            tc.tile_pool(name="poU", bufs=_b("POB", 1), space="PSUM")
        )

        # ---- constants into SBUF (once) ----
        cAW = []
        cEA = []
        for ci, (co, cn) in enumerate(_CHUNKS):
            t = cpool.tile([cn, H, N], DT, name=f"cAW{ci}")
            nc.sync.dma_start(t[:], AWc[co : co + cn])
            cAW.append(t)
            t = cpool.tile([cn, 2, 2 * N], DT, name=f"cEA{ci}")
            nc.sync.dma_start(t[:], EAc[co : co + cn])
            cEA.append(t)
        cK2 = cpool.tile([128, H, N], DT, name="cK2")
        nc.sync.dma_start(cK2[:], K2[:])
        cK3 = cpool.tile([128, N], DT, name="cK3")
        nc.sync.dma_start(cK3[64:128, :], K3HI[:])
        cEAT = []
        for ci, (co, cn) in enumerate(_CHUNKS):
            t = cpool.tile([cn, N], DT, name=f"cEAT{ci}")
            nc.sync.dma_start(t[:], EAT3[co : co + cn])
            cEAT.append(t)
        cTA = cpool.tile([FA, H, FA], DT, name="cTA")
        nc.sync.dma_start(cTA[:], TA[:])

        # ---- per batch item ----
        BG = min(_b("BG", 8), bpc)   # input DMA batching
        OG = min(_b("OG", 4), bpc)   # output DMA batching
        xg = [None, None]
        sog = [None, None]
        for b in range(bpc):
            # Group-load BG items of X_aug per chunk in one DMA. Tile layout
            # (cn, FA + BG*FA): cols [FA + g*FA, FA + (g+1)*FA) hold item g's
            # [features | ones]; the leading FA cols are zeroed so the M=128
            # "high-half" lhsT window (64 don't-care cols before the features)
            # is in-bounds for g=0.
            if b % BG == 0:
                ng = min(BG, bpc - b)
                src = Xa[b : b + ng].rearrange("g n f -> n g f")
                xg = []
                for ci, (co, cn) in enumerate(_CHUNKS):
                    t = xpool.tile([cn, (BG + 1) * FA], DT, tag=f"xa{ci}")
                    nc.vector.memset(t[:, 0:FA], 0.0)
                    nc.sync.dma_start(
                        t[:, FA : (ng + 1) * FA].rearrange(
                            "n (g f) -> n g f", f=FA
                        ),
                        src[co : co + cn],
                    )
                    xg.append(t)
            g = b % BG
            xa = [t[:, (g + 1) * FA - 64 : (g + 2) * FA] for t in xg]
            # xa[ci] is a (cn, 129) window: col 64 starts the features, so
            #   h-odd M=128 lhsT -> xa[:, 0:128]
            #   h-even M=64 lhsT -> xa[:, 64:128]
            #   step4 (f+ones)   -> xa[:, 64:129]

            # step1: featT (f, m) for 4 heads packed into one PSUM tile
            # layout [128, 2, N]: partition half = h%2 (0-63 even h, 64-127 odd),
            # free slot = h//2. Odd head first (M=128, writes garbage to the low
            # half), even head second (M=64, overwrites the low half).
            fAB = pf.tile([128, 2, 256], DT, tag="fp")
            for h in (1, 0, 3, 2):
                for ci, (co, cn) in enumerate(_CHUNKS):
                    if h % 2:
                        lhsT = xa[ci][:, 0:128]
                        out = fAB[:, h // 2, 0:N]
                    else:
                        lhsT = xa[ci][:, 64:128]
                        out = fAB[0:64, h // 2, 0:N]
                    nc.tensor.matmul(
                        out,
                        lhsT=lhsT,
                        rhs=cAW[ci][:, h, :],
                        start=(ci == 0),
                        stop=(ci == 1),
                    )
            fs = fspool.tile([128, 2, N], DT, tag="fs")
            if os.environ.get("FCE", "1") == "1":
                nc.scalar.copy(fs[:], fAB[:, :, 0:N])
            else:
                nc.vector.tensor_copy(fs[:], fAB[:, :, 0:N])
            if stage <= 1:
                for ci, (ro, rn) in enumerate(_CHUNKS):
                    sO = opool.tile([rn, OUTC], DT, tag=f"sO{ci}")
                    nc.vector.memset(sO[:], 0.0)
                    nc.vector.tensor_copy(sO[:, 0:398], fs[0:rn, :, 0:199])
                    nc.sync.dma_start(O[b, ro : ro + rn], sO[:])
                continue

            # step2: denseT (c, r) -- kernel[h] stationary; h pairs share the
            # PE via disjoint row groups (even h rows 0-63, odd h rows 64-127)
            # head h -> PSUM slot s=h%2 (=bank, so the row-split pair h0/h1
            # writes disjoint banks and may run concurrently), col block k=h//2
            eT = []
            for ci, (co, cn) in enumerate(_CHUNKS):
                t = pd.tile([cn, 2, 512], DT, tag="dnu", name=f"dT{ci}")
                for h in range(H):
                    pr = 64 * (h % 2)
                    nc.tensor.matmul(
                        t[:, h % 2, 199 * (h // 2) : 199 * (h // 2) + N],
                        lhsT=cK2[pr : pr + 64, h, co : co + cn],
                        rhs=fs[pr : pr + 64, h // 2, :],
                        start=True,
                        stop=True,
                        tile_position=(pr, 0),
                    )
                # exp + adjacency mask (+bias1 folded into EA)
                e = epool.tile([cn, 2, 2 * N], DT, tag=f"eT{ci}")
                eT.append(e)
                if os.environ.get("ESPLIT", "0") == "1":
                    for sl in range(2):
                        nc.scalar.activation(
                            e[:, sl, :], t[:, sl, 0 : 2 * N], AF.Exp
                        )
                        nc.vector.tensor_tensor(
                            e[:, sl, :], e[:, sl, :], cEA[ci][:, sl, :], ALU.mult
                        )
                else:
                    nc.scalar.activation(e[:], t[:, :, 0 : 2 * N], AF.Exp)
                    nc.vector.tensor_tensor(e[:], e[:], cEA[ci][:], ALU.mult)
            if stage <= 2:
                for ci, (ro, rn) in enumerate(_CHUNKS):
                    sO = opool.tile([rn, OUTC], DT, tag=f"sO{ci}")
                    nc.vector.tensor_copy(sO[:, 0:398], eT[ci][:, 0, :])
                    nc.vector.tensor_copy(sO[:, 398:OUTC], eT[ci][:, 1, 0:57])
                    nc.sync.dma_start(O[b, ro : ro + rn], sO[:])
                continue

            # step4: nodeUT (65, r) + denominator row (65th row via ones col)
            nU = pd.tile([65, 4, 256], DT, tag="dnu", name="nU")
            for h in range(H):
                for ci, (co, cn) in enumerate(_CHUNKS):
                    nc.tensor.matmul(
                        nU[:, h, 0:N],
                        lhsT=xa[ci][:, 64:129],
                        rhs=eT[ci][:, h % 2, 199 * (h // 2) : 199 * (h // 2) + N],
                        start=(ci == 0),
                        stop=(ci == 1),
                    )
            nS = nspool.tile([65, H, N], DT, tag="nS")
            if os.environ.get("NCE", "0") == "1":
                nc.scalar.copy(nS[:], nU[:, :, 0:N])
            else:
                nc.vector.tensor_copy(nS[:], nU[:, :, 0:N])
            if stage <= 3:
                for ci, (ro, rn) in enumerate(_CHUNKS):
                    sO = opool.tile([rn, OUTC], DT, tag=f"sO{ci}")
                    nc.vector.memset(sO[:], 0.0)
                    nc.vector.tensor_copy(sO[0:65, 0:398], nS[:, 0:2, :])
                    nc.sync.dma_start(O[b, ro : ro + rn], sO[:])
                continue

            # head-3 dense_rc into its own banks (fnu slot, free after nS copy)
            # so its matmul (PE rows 64-127) never shares a PSUM bank with the
            # concurrently-running step5 matmuls (rows 0-64)
            pR = pf.tile([128, 2, 256], DT, tag="fp")
            for ci, (ro, rn) in enumerate(_CHUNKS):
                nc.tensor.matmul(
                    pR[0:rn, ci, 0:N],
                    lhsT=fs[64:128, 1, ro : ro + rn],
                    rhs=cK3[64:128, :],
                    start=True,
                    stop=True,
                    tile_position=(64, 0),
                )

            # step5 per r-chunk; outputs staged in OG-item groups and DMA'd
            # out with one descriptor set per group per chunk
            go = b % OG
            if go == 0:
                sog = [
                    opool.tile([rn, OG, OUTC], DT, tag=f"sO{ci}", name=f"sOg{ci}")
                    for ci, (ro, rn) in enumerate(_CHUNKS)
                ]
            for ci, (ro, rn) in enumerate(_CHUNKS):
                oUF = po.tile(
                    [rn, 260], DT,
                    tag="oU" if os.environ.get("OUM", "0") == "1" else f"oUF{ci}",
                    bufs=2 if os.environ.get("OUM", "0") == "1" else None,
                )
                for h in range(H):
                    nc.tensor.matmul(
                        oUF[:, 65 * h : 65 * h + 65],
                        lhsT=nS[:, h, ro : ro + rn],
                        rhs=cTA[:, h, :],
                        start=True,
                        stop=True,
                    )

                # 1/s for all 4 heads: s sits at col 64 of each 65-wide block
                rec = rpool.tile([rn, H], DT, tag=f"rec{ci}")
                oUh = oUF[:].rearrange("p (h j) -> p h j", j=65)
                nc.vector.reciprocal(rec[:], oUh[:, :, 64])

                sO = sog[ci][:, go]
                # head outputs normalized by 1/s (free-dim broadcast of rec)
                nc.vector.tensor_tensor(
                    sO[:, 0 : H * U].rearrange("p (h u) -> p h u", u=U),
                    oUh[:, :, 0:U],
                    rec[:, :, None].to_broadcast((rn, H, U)),
                    ALU.mult,
                )
                # head-3 mask: exp, adjacency mask, normalize
                eR = erpool.tile([rn, N], DT, tag=f"eR{ci}")
                nc.scalar.activation(eR[:], pR[0:rn, ci, 0:N], AF.Exp)
                nc.vector.tensor_tensor(eR[:], eR[:], cEAT[ci][:], ALU.mult)
                nc.vector.tensor_scalar_mul(sO[:, H * U : OUTC], eR[:], rec[:, 3:4])

                if go == OG - 1 or b == bpc - 1:
                    ng = go + 1
                    nc.sync.dma_start(
                        O[b - go : b + 1, ro : ro + rn].rearrange("g n c -> n g c"),
                        sog[ci][:, 0:ng],
                    )


def build_nc(bpc: int = BPC, num_devices: int = NCORES) -> bass.Bass:
    nc = bacc.Bacc(
        "TRN2",
        target_bir_lowering=False,
        debug=False,
        num_devices=num_devices,
    )
    io = {
        "Xa": nc.dram_tensor("Xa", [bpc, N, FA], DT, kind="ExternalInput").ap(),
        "AWc": nc.dram_tensor("AWc", [N, H, N], DT, kind="ExternalInput").ap(),
        "K2": nc.dram_tensor("K2", [128, H, N], DT, kind="ExternalInput").ap(),
        "K3HI": nc.dram_tensor("K3HI", [F, N], DT, kind="ExternalInput").ap(),
        "EAc": nc.dram_tensor("EAc", [N, 2, 2 * N], DT, kind="ExternalInput").ap(),
        "EAT3": nc.dram_tensor("EAT3", [N, N], DT, kind="ExternalInput").ap(),
        "TA": nc.dram_tensor("TA", [FA, H, FA], DT, kind="ExternalInput").ap(),
        "O": nc.dram_tensor("O", [bpc, N, OUTC], DT, kind="ExternalOutput").ap(),
    }
    with tile.TileContext(nc) as tc:
        _build_kernel(nc, tc, io, bpc=bpc)
    nc.compile()
    return nc


def _prep_weights(A, W, kernel, T, bias1, bias2):
    """Host-side constant prep (tiny tensors)."""
    A = np.asarray(A, np.float32)
    W = np.asarray(W, np.float32)
    kernel = np.asarray(kernel, np.float32)
    T = np.asarray(T, np.float32)
    bias1 = np.asarray(bias1, np.float32)
    bias2 = np.asarray(bias2, np.float32)

    AW = A[None, :, :] * W  # (H, n, m)
    AWc = np.ascontiguousarray(AW.transpose(1, 0, 2))  # [n, h, m]

    Kf = kernel  # (H, F, N): [h, f, c]
    K1 = np.ascontiguousarray(Kf.transpose(1, 0, 2))  # [f, h, c]
    K2 = np.concatenate([K1, K1], axis=0)  # duplicate f-rows for PE rows 64-127
    K3HI = np.ascontiguousarray(Kf[3])  # (F, N)

    eb1 = np.exp(bias1)  # (H, N) over c
    # EA[c, h, r] = A[r, c] * exp(bias1[h, c]); packed as [c, s, k*199 + r]
    # with h = 2k + s (s = slot/bank, k = col block)
    EA = A.T[:, None, :] * eb1.T[:, :, None]  # (c, h, r)
    EAc = np.ascontiguousarray(
        EA.reshape(N, 2, 2, N).transpose(0, 2, 1, 3).reshape(N, 2, 2 * N)
    )
    # EAT3[r, c] = A[r, c] * exp(bias1[3, c])
    EAT3 = np.ascontiguousarray(A * eb1[3][None, :])

    # T_aug2[h]: (65, 65): rows 0-63 = T[h], row 64 = [bias2[h], 1.0-at-col-64]
    TA = np.zeros((FA, H, FA), np.float32)
    TA[:F, :, :U] = T.transpose(1, 0, 2)
    TA[F, :, :U] = bias2
    TA[F, :, U] = 1.0
    return dict(AWc=AWc, K2=K2, K3HI=K3HI, EAc=EAc, EAT3=EAT3, TA=TA)


_CACHED = None


def _get_executable():
    """Build the Bass module once and wrap it in a reusable sharded jax jit.

    Mirrors concourse.bass2jax.run_bass_via_pjrt's multi-core path, but caches
    the jitted callable so repeated kernel() calls skip re-lowering the BIR.
    """
    global _CACHED
    if _CACHED is not None:
        return _CACHED

    import jax
    from jax.sharding import Mesh, PartitionSpec
    from jax.experimental.shard_map import shard_map

    import concourse.mybir as _mybir
    from concourse import bass2jax

    bass2jax.install_neuronx_cc_hook()
    nc = build_nc()

    partition_name = (
        nc.partition_id_tensor.name if nc.partition_id_tensor else None
    )
    in_names, out_names, out_avals = [], [], []
    for alloc in nc.m.functions[0].allocations:
        if not isinstance(alloc, _mybir.MemoryLocationSet):
            continue
        name = alloc.memorylocations[0].name
        if alloc.kind == "ExternalInput":
            if name != partition_name:
                in_names.append(name)
        elif alloc.kind == "ExternalOutput":
            out_names.append(name)
            out_avals.append(
                jax.core.ShapedArray(
                    tuple(alloc.tensor_shape), _mybir.dt.np(alloc.dtype)
                )
            )
    n_params = len(in_names)
    n_outs = len(out_avals)
    all_in_names = list(in_names) + list(out_names)
    if partition_name is not None:
        all_in_names.append(partition_name)

    def _body(*args):
        operands = list(args)
        if partition_name is not None:
            operands.append(bass2jax.partition_id_tensor())
        outs = bass2jax._bass_exec_p.bind(
            *operands,
            out_avals=tuple(out_avals),
            in_names=tuple(all_in_names),
            out_names=tuple(out_names),
            lowering_input_output_aliases=(),
            sim_require_finite=True,
            sim_require_nnan=True,
            nc=nc,
        )
        return tuple(outs)

    devices = jax.devices()[:NCORES]
    mesh = Mesh(np.asarray(devices), ("core",))
    in_specs = (PartitionSpec("core"),) * (n_params + n_outs)
    out_specs = (PartitionSpec("core"),) * n_outs
    sharded = jax.jit(
        shard_map(
            _body, mesh=mesh, in_specs=in_specs, out_specs=out_specs,
            check_rep=False,
        ),
        donate_argnums=tuple(range(n_params, n_params + n_outs)),
        keep_unused=True,
    )
    _CACHED = (sharded, in_names, out_names, out_avals, jax, mesh)
    return _CACHED


def _stage_inputs(inputs):
    X = np.asarray(inputs["X"], np.float32)
    consts = _prep_weights(
        inputs["A"], inputs["W"], inputs["kernel"], inputs["T"],
        inputs["bias1"], inputs["bias2"],
    )
    Xa = np.concatenate([X, np.ones((B, N, 1), np.float32)], axis=2)
    per_core = {"Xa": np.ascontiguousarray(Xa)}  # (B, N, FA): axis0 shards
    for k, v in consts.items():
        per_core[k] = np.concatenate([v] * NCORES, axis=0)
    return per_core


def _run(staged):
    sharded, in_names, out_names, out_avals, jax, mesh = _get_executable()
    concat_in = [staged[nm] for nm in in_names]
    zeros = [
        np.zeros((NCORES * a.shape[0], *a.shape[1:]), a.dtype) for a in out_avals
    ]
    out_arrs = sharded(*concat_in, *zeros)
    return np.asarray(out_arrs[out_names.index("O")])


def kernel(**inputs) -> np.ndarray:
    staged = _stage_inputs(inputs)
    out = _run(staged)  # (NCORES*BPC, N, OUTC) = (B, N, OUTC)
    return out



# revision 33
# speedup vs baseline: 166.8783x; 166.8783x over previous
"""Trainium2 Bass kernel for DGCRNNCell (nn_DGCRNNCell_21792664060192).

Computes, for each batch item b and head h over graph with N=199 nodes:
  feat   = einsum('nf,nm->mf', X[b], A*W[h])          (via featT = X^T-style chain)
  dense  = feat @ kernel[h] + bias1[h]
  mask   = softmax(dense - NEG*(1-A), axis=-1)        (adjacency-masked softmax)
  node   = mask @ X[b]
  out_h  = node @ T[h] + bias2[h]
  output[b] = concat([out_0..out_3 (r, 256)], mask_3 (r, 199))   -> (199, 455)

Sharding: pure data-parallel over batch (512 -> 64 per core x 8 cores).

Dataflow on device (per b), all matmul chains arranged so that no transpose
is ever needed (the contraction axis always lands on partitions):
  featT  (f=64, m)    = sum_n  Xb(n,f)^T ... lhsT=Xb chunk, rhs=AW[h] chunk
  denseT (c, r)       : lhsT=kernel[h](f,c-chunk), rhs=featT(f,r)
  expT   (c, r)       = exp(denseT) * EA[h]   where EA[h][c,r] = A[r,c]*e^{bias1[h,c]}
  nodeUT (65, r)      : lhsT=Xb_aug(c,65) (ones col -> row 64 = softmax denom s[r])
  outU   (r, 65h+j)   : lhsT=nodeUT(65, r-chunk), rhs=T_aug[h](65,65)
                        col 64 of each head block = s[r]; row 64 of T_aug = bias2
  out    (r, h*64+u)  = outU * (1/s[r])   (per-partition broadcast multiply)
Head-3 mask output is computed in (r, c) orientation directly:
  dense_rc: lhsT=featT_3(f, r-chunk), rhs=kernel[3](f, c); exp; * EAT3; * 1/s3.
"""

import numpy as np

import concourse.bass as bass
import concourse.mybir as mybir
import concourse.tile as tile
from concourse import bacc
from concourse.bass_utils import run_bass_kernel_spmd

B, N, F, U, H = 512, 199, 64, 64, 4
NCORES = 8
BPC = B // NCORES  # 64 batch items per core
P0 = 128
P1 = N - P0  # 71
FA = F + 1  # X augmented with ones column
OUTC = H * U + N  # 455
DT = mybir.dt.float32
BF = mybir.dt.bfloat16
AF = mybir.ActivationFunctionType
ALU = mybir.AluOpType

_CHUNKS = ((0, P0), (P0, P1))  # (offset, size) along the N(=c or r) axis


def _build_kernel(nc: bass.Bass, tc: "tile.TileContext", io: dict, bpc: int = BPC):
    import os
    from contextlib import ExitStack

    stage = int(os.environ.get("KSTAGE", "9"))

    Xa, AWc, K2, EAc, TA, ID, O = (
        io["Xa"], io["AWc"], io["K2"], io["EAc"], io["TA"], io["ID"], io["O"],
    )

    def _b(name, default):
        return int(os.environ.get(name, str(default)))

    with ExitStack() as ctx:
        cpool = ctx.enter_context(tc.tile_pool(name="consts", bufs=1))
        xpool = ctx.enter_context(tc.tile_pool(name="xa", bufs=_b("XB", 3)))
        fspool = ctx.enter_context(tc.tile_pool(name="fs", bufs=_b("FSB", 2)))
        epool = ctx.enter_context(tc.tile_pool(name="expT", bufs=_b("EB", 2)))
        nspool = ctx.enter_context(tc.tile_pool(name="nS", bufs=_b("NSB", 2)))
        rpool = ctx.enter_context(tc.tile_pool(name="rec", bufs=_b("RB", 2)))
        opool = ctx.enter_context(tc.tile_pool(name="sO", bufs=_b("OB", 3)))
        pf = ctx.enter_context(
            tc.tile_pool(name="pfnu", bufs=_b("FNB", 2), space="PSUM")
        )
        pd = ctx.enter_context(
            tc.tile_pool(name="pdnu", bufs=_b("DTB", 2), space="PSUM")
        )
        po = ctx.enter_context(
            tc.tile_pool(name="poU", bufs=_b("POB", 1), space="PSUM")
        )

        # ---- constants into SBUF (once) ----
        skipc = os.environ.get("SKIPC", "0") == "1"  # timing ablation
        cAW = []
        cEA = []
        for ci, (co, cn) in enumerate(_CHUNKS):
            t = cpool.tile([cn, H, N], BF, name=f"cAW{ci}")
            if not skipc:
                nc.sync.dma_start(t[:], AWc[co : co + cn])
            cAW.append(t)
            t = cpool.tile([cn, 2, 2 * N], BF, name=f"cEA{ci}")
            if not skipc:
                nc.sync.dma_start(t[:], EAc[co : co + cn])
            cEA.append(t)
        cK2 = cpool.tile([128, H, N], BF, name="cK2")
        cTA = cpool.tile([FA, H, FA], BF, name="cTA")
        cID = cpool.tile([128, 128], BF, name="cID")
        if not skipc:
            nc.sync.dma_start(cK2[:], K2[:])
            nc.sync.dma_start(cTA[:], TA[:])
            nc.sync.dma_start(cID[:], ID[:])

        # ---- per batch item ----
        BG = min(_b("BG", 8), bpc)   # input DMA batching
        OG = min(_b("OG", 4), bpc)   # output DMA batching
        xg = [None, None]
        sog = [None, None]
        for b in range(bpc):
            # Group-load BG items of X_aug per chunk in one DMA. Tile layout
            # (cn, FA + BG*FA): cols [FA + g*FA, FA + (g+1)*FA) hold item g's
            # [features | ones]; the leading FA cols are zeroed so the M=128
            # "high-half" lhsT window (64 don't-care cols before the features)
            # is in-bounds for g=0.
            if b % BG == 0:
                ng = min(BG, bpc - b)
                src = Xa[b : b + ng].rearrange("g n f -> n g f")
                xg = []
                for ci, (co, cn) in enumerate(_CHUNKS):
                    t = xpool.tile([cn, BG * FA], BF, tag=f"xa{ci}")
                    if os.environ.get("SKIPX", "0") != "1":  # timing ablation
                        nc.sync.dma_start(
                            t[:, 0 : ng * FA].rearrange("n (g f) -> n g f", f=FA),
                            src[co : co + cn],
                        )
                    xg.append(t)
            g = b % BG
            xa = [t[:, g * FA : (g + 1) * FA] for t in xg]
            # xa[ci] is a (cn, 65) window: [f0..f63 | ones]
            #   step1/2 lhsT -> xa[:, 0:64]; step4 lhsT -> xa[:, 0:65]

            # step1: featT (f, m) for 4 heads packed into one PSUM tile
            # layout [128, 2, N]: partition half = h%2 (0-63 even h, 64-127
            # odd), free slot = h//2. Each head is an M=64 matmul; the
            # even/odd pair targets disjoint PE column groups
            # (tile_position col 0 / 64) so the pair runs concurrently.
            fAB = pf.tile([128, 2, 256], DT, tag="fp")
            for hp in range(2):
                for ci, (co, cn) in enumerate(_CHUNKS):
                    for h in (2 * hp, 2 * hp + 1):
                        pr = 64 * (h % 2)
                        nc.tensor.matmul(
                            fAB[pr : pr + 64, h // 2, 0:N],
                            lhsT=xa[ci][:, 0:64],
                            rhs=cAW[ci][:, h, :],
                            start=(ci == 0),
                            stop=(ci == 1),
                            tile_position=(0, pr),
                        )
            fs = fspool.tile([128, 2, N], BF, tag="fs")
            if os.environ.get("FCE", "1") == "1":
                nc.scalar.copy(fs[:], fAB[:, :, 0:N])
            else:
                nc.vector.tensor_copy(fs[:], fAB[:, :, 0:N])
            if stage <= 1:
                for ci, (ro, rn) in enumerate(_CHUNKS):
                    sO = opool.tile([rn, OUTC], DT, tag=f"sO{ci}")
                    nc.vector.memset(sO[:], 0.0)
                    nc.vector.tensor_copy(sO[:, 0:398], fs[0:rn, :, 0:199])
                    nc.sync.dma_start(O[b, ro : ro + rn], sO[:])
                continue

            # step2: denseT (c, r) -- kernel[h] stationary; h pairs share the
            # PE via disjoint row groups (even h rows 0-63, odd h rows 64-127)
            # head h -> PSUM slot s=h%2 (=bank, so the row-split pair h0/h1
            # writes disjoint banks and may run concurrently), col block k=h//2.
            # The adjacency mask (-1e16 where A=0) plus bias1 is accumulated
            # into the same PSUM region first via an identity-weight matmul
            # streaming the precomputed MK constant, so exp's output is the
            # final masked e with no elementwise fixup pass.
            eT = []
            for ci, (co, cn) in enumerate(_CHUNKS):
                t = pd.tile([cn, 2, 512], DT, tag="dnu", name=f"dT{ci}")
                for sl in range(2):
                    nc.tensor.matmul(
                        t[:, sl, 0 : 2 * N],
                        lhsT=cID[0:cn, 0:cn],
                        rhs=cEA[ci][:, sl, :],
                        start=True,
                        stop=False,
                    )
                for h in range(H):
                    pr = 64 * (h % 2)
                    nc.tensor.matmul(
                        t[:, h % 2, 199 * (h // 2) : 199 * (h // 2) + N],
                        lhsT=cK2[pr : pr + 64, h, co : co + cn],
                        rhs=fs[pr : pr + 64, h // 2, :],
                        start=False,
                        stop=True,
                        tile_position=(pr, 0),
                    )
                e = epool.tile([cn, 2, 2 * N], BF, tag=f"eT{ci}")
                eT.append(e)
                nc.scalar.activation(e[:], t[:, :, 0 : 2 * N], AF.Exp)
            if stage <= 2:
                for ci, (ro, rn) in enumerate(_CHUNKS):
                    sO = opool.tile([rn, OUTC], DT, tag=f"sO{ci}")
                    nc.vector.tensor_copy(sO[:, 0:398], eT[ci][:, 0, :])
                    nc.vector.tensor_copy(sO[:, 398:OUTC], eT[ci][:, 1, 0:57])
                    nc.sync.dma_start(O[b, ro : ro + rn], sO[:])
                continue

            # step4: nodeUT (65, r) + denominator row (65th row via ones col)
            # One matmul per (slot, chunk): rhs = both heads of the slot.
            # Head h = 2k + s lands at nU[:, s, k*199 : k*199+199].
            nU = pd.tile([65, 2, 512], DT, tag="dnu", name="nU")
            for sl in range(2):
                for ci, (co, cn) in enumerate(_CHUNKS):
                    nc.tensor.matmul(
                        nU[:, sl, 0 : 2 * N],
                        lhsT=xa[ci][:, 0:65],
                        rhs=eT[ci][:, sl, :],
                        start=(ci == 0),
                        stop=(ci == 1),
                    )
            nS = nspool.tile([65, 2, 2 * N], BF, tag="nS")
            if os.environ.get("NCE", "0") == "1":
                nc.scalar.copy(nS[:], nU[:, :, 0 : 2 * N])
            else:
                nc.vector.tensor_copy(nS[:], nU[:, :, 0 : 2 * N])
            if stage <= 3:
                for ci, (ro, rn) in enumerate(_CHUNKS):
                    sO = opool.tile([rn, OUTC], DT, tag=f"sO{ci}")
                    nc.vector.memset(sO[:], 0.0)
                    nc.vector.tensor_copy(sO[0:65, 0:398], nS[:, :, 0:N])
                    nc.sync.dma_start(O[b, ro : ro + rn], sO[:])
                continue

            # head-3 mask transposed into (r, c) orientation for the output:
            # PE-transpose of the already-masked e3 (slot 1, col block 1)
            pR = pf.tile([128, 2, 256], BF, tag="fp")
            for rj, (ro, rn) in enumerate(_CHUNKS):
                for ci, (co, cn) in enumerate(_CHUNKS):
                    nc.tensor.transpose(
                        pR[0:rn, rj, co : co + cn],
                        in_=eT[ci][:, 1, N + ro : N + ro + rn],
                        identity=cID[0:cn, 0:cn],
                    )

            # step5 per r-chunk; outputs staged in OG-item groups and DMA'd
            # out with one descriptor set per group per chunk
            go = b % OG
            if go == 0:
                sog = [
                    opool.tile([rn, OG, OUTC], DT, tag=f"sO{ci}", name=f"sOg{ci}")
                    for ci, (ro, rn) in enumerate(_CHUNKS)
                ]
            for ci, (ro, rn) in enumerate(_CHUNKS):
                oUF = po.tile(
                    [rn, 260], DT,
                    tag="oU" if os.environ.get("OUM", "0") == "1" else f"oUF{ci}",
                    bufs=2 if os.environ.get("OUM", "0") == "1" else None,
                )
                for h in range(H):
                    nc.tensor.matmul(
                        oUF[:, 65 * h : 65 * h + 65],
                        lhsT=nS[:, h % 2, 199 * (h // 2) + ro : 199 * (h // 2) + ro + rn],
                        rhs=cTA[:, h, :],
                        start=True,
                        stop=True,
                    )

                # 1/s for all 4 heads: s sits at col 64 of each 65-wide block
                rec = rpool.tile([rn, H], DT, tag=f"rec{ci}")
                oUh = oUF[:].rearrange("p (h j) -> p h j", j=65)
                nc.vector.reciprocal(rec[:], oUh[:, :, 64])

                sO = sog[ci][:, go]
                # head outputs normalized by 1/s (free-dim broadcast of rec)
                nc.vector.tensor_tensor(
                    sO[:, 0 : H * U].rearrange("p (h u) -> p h u", u=U),
                    oUh[:, :, 0:U],
                    rec[:, :, None].to_broadcast((rn, H, U)),
                    ALU.mult,
                )
                # head-3 mask: transposed-masked e3, normalized
                nc.vector.tensor_scalar_mul(
                    sO[:, H * U : OUTC], pR[0:rn, ci, 0:N], rec[:, 3:4]
                )

                if go == OG - 1 or b == bpc - 1:
                    ng = go + 1
                    if os.environ.get("SKIPO", "0") != "1":  # timing ablation
                        nc.sync.dma_start(
                            O[b - go : b + 1, ro : ro + rn].rearrange(
                                "g n c -> n g c"
                            ),
                            sog[ci][:, 0:ng],
                        )


def build_nc(
    bpc: int = BPC, num_devices: int = NCORES, repeat: int = 1
) -> bass.Bass:
    nc = bacc.Bacc(
        "TRN2",
        target_bir_lowering=False,
        debug=False,
        num_devices=num_devices,
    )
    io = {
        "Xa": nc.dram_tensor("Xa", [bpc, N, FA], BF, kind="ExternalInput").ap(),
        "AWc": nc.dram_tensor("AWc", [N, H, N], BF, kind="ExternalInput").ap(),
        "K2": nc.dram_tensor("K2", [128, H, N], BF, kind="ExternalInput").ap(),
        "EAc": nc.dram_tensor("EAc", [N, 2, 2 * N], BF, kind="ExternalInput").ap(),
        "TA": nc.dram_tensor("TA", [FA, H, FA], BF, kind="ExternalInput").ap(),
        "ID": nc.dram_tensor("ID", [128, 128], BF, kind="ExternalInput").ap(),
        "O": nc.dram_tensor("O", [bpc, N, OUTC], DT, kind="ExternalOutput").ap(),
    }
    with tile.TileContext(nc) as tc:
        if repeat == 1:
            _build_kernel(nc, tc, io, bpc=bpc)
        else:
            # Timing-only variant: re-run the identical workload `repeat`
            # times in a hardware loop (same output written each pass) so
            # per-dispatch tunnel latency can be amortized out of the
            # hardware-time measurement.
            with tc.For_i(0, repeat, 1):
                _build_kernel(nc, tc, io, bpc=bpc)
    nc.compile()
    return nc


def _prep_weights(A, W, kernel, T, bias1, bias2):
    """Host-side constant prep (tiny tensors)."""
    A = np.asarray(A, np.float32)
    W = np.asarray(W, np.float32)
    kernel = np.asarray(kernel, np.float32)
    T = np.asarray(T, np.float32)
    bias1 = np.asarray(bias1, np.float32)
    bias2 = np.asarray(bias2, np.float32)

    AW = A[None, :, :] * W  # (H, n, m)
    AWc = np.ascontiguousarray(AW.transpose(1, 0, 2))  # [n, h, m]

    Kf = kernel  # (H, F, N): [h, f, c]
    K1 = np.ascontiguousarray(Kf.transpose(1, 0, 2))  # [f, h, c]
    K2 = np.concatenate([K1, K1], axis=0)  # duplicate f-rows for PE rows 64-127

    # MK[c, h, r] = bias1[h, c] - 1e16 * (1 - A[r, c]): additive logit fixup
    # (adjacency mask + bias1) accumulated into dense via identity matmul;
    # packed as [c, s, k*199 + r] with h = 2k + s (s = slot/bank, k = block)
    MK = bias1.T[:, :, None] - 1e16 * (1.0 - A.T[:, None, :])  # (c, h, r)
    EAc = np.ascontiguousarray(
        MK.reshape(N, 2, 2, N).transpose(0, 2, 1, 3).reshape(N, 2, 2 * N)
    )

    # T_aug2[h]: (65, 65): rows 0-63 = T[h], row 64 = [bias2[h], 1.0-at-col-64]
    TA = np.zeros((FA, H, FA), np.float32)
    TA[:F, :, :U] = T.transpose(1, 0, 2)
    TA[F, :, :U] = bias2
    TA[F, :, U] = 1.0
    import ml_dtypes

    bf = ml_dtypes.bfloat16
    return dict(
        AWc=AWc.astype(bf), K2=K2.astype(bf), EAc=EAc.astype(bf),
        TA=TA.astype(bf), ID=np.eye(128, dtype=bf),
    )


_CACHED = {}


def _get_executable(repeat: int = 1):
    """Build the Bass module once and wrap it in a reusable sharded jax jit.

    Mirrors concourse.bass2jax.run_bass_via_pjrt's multi-core path, but caches
    the jitted callable so repeated kernel() calls skip re-lowering the BIR.
    """
    if repeat in _CACHED:
        return _CACHED[repeat]

    import jax
    from jax.sharding import Mesh, PartitionSpec
    from jax.experimental.shard_map import shard_map

    import concourse.mybir as _mybir
    from concourse import bass2jax

    bass2jax.install_neuronx_cc_hook()
    nc = build_nc(repeat=repeat)

    partition_name = (
        nc.partition_id_tensor.name if nc.partition_id_tensor else None
    )
    in_names, out_names, out_avals = [], [], []
    for alloc in nc.m.functions[0].allocations:
        if not isinstance(alloc, _mybir.MemoryLocationSet):
            continue
        name = alloc.memorylocations[0].name
        if alloc.kind == "ExternalInput":
            if name != partition_name:
                in_names.append(name)
        elif alloc.kind == "ExternalOutput":
            out_names.append(name)
            out_avals.append(
                jax.core.ShapedArray(
                    tuple(alloc.tensor_shape), _mybir.dt.np(alloc.dtype)
                )
            )
    n_params = len(in_names)
    n_outs = len(out_avals)
    all_in_names = list(in_names) + list(out_names)
    if partition_name is not None:
        all_in_names.append(partition_name)

    def _body(*args):
        operands = list(args)
        if partition_name is not None:
            operands.append(bass2jax.partition_id_tensor())
        outs = bass2jax._bass_exec_p.bind(
            *operands,
            out_avals=tuple(out_avals),
            in_names=tuple(all_in_names),
            out_names=tuple(out_names),
            lowering_input_output_aliases=(),
            sim_require_finite=True,
            sim_require_nnan=True,
            nc=nc,
        )
        return tuple(outs)

    devices = jax.devices()[:NCORES]
    mesh = Mesh(np.asarray(devices), ("core",))
    in_specs = (PartitionSpec("core"),) * (n_params + n_outs)
    out_specs = (PartitionSpec("core"),) * n_outs
    sharded = jax.jit(
        shard_map(
            _body, mesh=mesh, in_specs=in_specs, out_specs=out_specs,
            check_rep=False,
        ),
        donate_argnums=tuple(range(n_params, n_params + n_outs)),
        keep_unused=True,
    )
    _CACHED[repeat] = (sharded, in_names, out_names, out_avals, jax, mesh)
    return _CACHED[repeat]


def _stage_inputs(inputs):
    import ml_dtypes

    X = np.asarray(inputs["X"], np.float32)
    consts = _prep_weights(
        inputs["A"], inputs["W"], inputs["kernel"], inputs["T"],
        inputs["bias1"], inputs["bias2"],
    )
    Xa = np.concatenate([X, np.ones((B, N, 1), np.float32)], axis=2)
    # (B, N, FA): axis0 shards
    per_core = {"Xa": np.ascontiguousarray(Xa.astype(ml_dtypes.bfloat16))}
    for k, v in consts.items():
        per_core[k] = np.concatenate([v] * NCORES, axis=0)
    return per_core


def _run(staged):
    sharded, in_names, out_names, out_avals, jax, mesh = _get_executable()
    concat_in = [staged[nm] for nm in in_names]
    zeros = [
        np.zeros((NCORES * a.shape[0], *a.shape[1:]), a.dtype) for a in out_avals
    ]
    out_arrs = sharded(*concat_in, *zeros)
    return np.asarray(out_arrs[out_names.index("O")])


def kernel(**inputs) -> np.ndarray:
    staged = _stage_inputs(inputs)
    out = _run(staged)  # (NCORES*BPC, N, OUTC) = (B, N, OUTC)
    return out

